# revision 36
# baseline (speedup 1.0000x reference)
"""Trainium2 Bass kernel: 32-bit soft-logic comparator (A > B, A == B).

Inputs A, B: [2_000_000, 32] float32 with values in {0.0, 1.0}, MSB first.
Outputs: (a_gt_b, a_eq_b), each [2_000_000, 1] float32 in {0.0, 1.0}.

Algorithm (exact, replaces the reference's prefix-product ladder):
  e_i = (a_i - b_i) * 2^(31-i)   in {0, +-2^(31-i)}
  V   = sum_i e_i  =  int32(A) - int32(B)
  a_gt_b = (V > 0), a_eq_b = (V == 0)
The sum is evaluated as a pairwise tree per 8-bit segment (every partial
sum inside segment j is an integer multiple of 2^(24-8j) with < 2^8
octave span => exactly representable in bf16), then the four segment
sums combine in fp32: V1 = s0+s1 (16 significant bits, exact),
V0 = s2+s3 (<= 65535, exact), V = V1+V0 (one rounded add of a multiple
of 2^16 plus a < 2^16 term: sign exact, zero iff exact zero).

Input staging: the values {0, 2^k} are exactly representable in
bfloat16, so the host encodes each input column in a fixed pre-scaled
bf16 fixed-point form (bit * 2^(31-i), a per-element lossless map).
The device then streams half the HBM bytes (memory-bound regime) and
the whole comparator pipeline (subtract, 3-level tree, combine,
compares) runs on-device on the DVE at its 2x bf16 rate. A plain
{0,1}-bf16 staging variant (device-side weight multiply) is kept behind
cfg["prescale"]=False.

Engine assignment: everything on DVE. Offloading tree levels to GpSimd
was measured and REGRESSES: concurrent GpSimd tensor ops slow DVE ops
sharing SBUF by 1.7x-9x (bank contention), and GpSimd small ops carry
~2us fixed overhead. gt/eq go into one interleaved [rows, 2] tile from
a dedicated deep pool (8 bufs) so store-queue backpressure never stalls
the compares; one DMA store per tile.

Sharding: data parallel along dim 0 across 8 cores, 250112 = 128*1954
rows/core; only the last core is zero-padded (896 rows), dropped on
gather.

Per-core layout: SBUF tiles [128, k*32] where partition p holds k
consecutive rows; 15 tiles of k=128 plus one tail tile of k=34.
Measured on trn2 (NTFF profile): ~105-126 us/core vs 91 us bf16 DMA
floor and 92 us DVE busy; the f32-input baseline was ~202-212 us.
"""

import numpy as np

N = 2_000_000
BITS = 32
NCORES = 8
P = 128
ROWS_PER_CORE = 250_112          # 128 * 1954 >= 2_000_000 / 8
GROUPS = ROWS_PER_CORE // P      # 1954 rows per partition per core
K_MAIN = 128
KS = [K_MAIN] * (GROUPS // K_MAIN) + ([GROUPS % K_MAIN] if GROUPS % K_MAIN else [])
SEG = 8                          # bits per exact-sum segment (b1 variant)

_CACHE = {}


def _weight_row():
    # w_i = 2^(31-i), MSB first; powers of two, exact in bf16.
    return 2.0 ** (31 - np.arange(BITS, dtype=np.float64))


def _ks(cfg):
    km = cfg["kmain"]
    ks = [km] * (GROUPS // km) + ([GROUPS % km] if GROUPS % km else [])
    if cfg["ramp"]:
        ks = [32, km - 32] + ks[1:]
    return ks


def _emit_pass_b1(nc, pool, spool, opool, wt, a_dram, b_dram, o_dram, mybir,
                  cfg, dma_only=False):
    """bf16 pipeline: sub (DVE 2x), 3-level pairwise-sum tree,
    combine, compares; one interleaved [rows, 2] store per tile."""
    dt = mybir.dt
    Alu = mybir.AluOpType

    def eng(name):
        return {"vector": nc.vector, "gpsimd": nc.gpsimd,
                "scalar": nc.scalar, "sync": nc.sync}[name]

    row0 = 0
    for k in _ks(cfg):
        rows = P * k
        F = k * BITS
        if cfg["abt"] == "ilv":
            # A and B arrive byte-interleaved from the host: row layout
            # [a0..7, b0..7, a8..15, b8..15, ...]. The subtract's operand
            # streams are then 16 B apart (the same access shape as the
            # fast tree adds), out is compact and in natural bit order.
            ab = pool.tile([P, 2 * F], dt.bfloat16, tag="ab")
            abv = ab[:].rearrange("p (s x) -> p s x", x=16)
            a, b = abv[:, :, 0:8], abv[:, :, 8:16]
            iv = a_dram[:].flatten()[row0 * 2 * BITS:(row0 + rows) * 2 * BITS] \
                .rearrange("(p f) -> p f", p=P)
            h2 = 2 * F // 2
            nc.sync.dma_start(out=ab[:, :h2], in_=iv[:, :h2])
            nc.scalar.dma_start(out=ab[:, h2:], in_=iv[:, h2:])
        else:
            av = a_dram[:].flatten()[row0 * BITS:(row0 + rows) * BITS] \
                .rearrange("(p f) -> p f", p=P)
            bv = b_dram[:].flatten()[row0 * BITS:(row0 + rows) * BITS] \
                .rearrange("(p f) -> p f", p=P)
            if cfg["abt"]:
                # a and b side by side in one tile: the subtract's two
                # operand streams stay address-adjacent in SBUF.
                ab = pool.tile([P, 2 * F], dt.bfloat16, tag="ab")
                a, b = ab[:, :F], ab[:, F:]
            else:
                at = pool.tile([P, F], dt.bfloat16, tag="a")
                bt = pool.tile([P, F], dt.bfloat16, tag="b")
                a, b = at[:], bt[:]
            nc.sync.dma_start(out=a, in_=av)
            nc.scalar.dma_start(out=b, in_=bv)
        if dma_only:
            row0 += rows
            continue

        # d = a - b in {-1,0,1}; e = d * w with w_i = 2^(31-i). Both bf16
        # exact (d*w is a signed power of two), DVE 2x rate. Writing d over
        # the a operand keeps the op at two SBUF streams (measured ~2.2us
        # vs ~3.6us for a third fresh stream).
        if cfg["prescale"]:
            # Host ships A*w and B*w (w_i = 2^(31-i), both exact bf16
            # encodings of the bits): the subtract directly yields the
            # weighted difference e.
            e = pool.tile([P, F], dt.bfloat16, tag="e")
            nc.vector.tensor_tensor(e[:], a, b, Alu.subtract)
        else:
            if cfg["subout"] == "inplace" and cfg["abt"] is True:
                dd = a
            else:
                dt_ = pool.tile([P, F], dt.bfloat16, tag="d")
                dd = dt_[:]
            dv = dd.rearrange("p (s x) -> p s x", x=8) \
                if cfg["abt"] == "ilv" else dd
            nc.vector.tensor_tensor(dv, a, b, Alu.subtract)
            e = pool.tile([P, F], dt.bfloat16, tag="e")
            nc.vector.tensor_tensor(e[:], dd, wt[:, :F], Alu.mult)

        # Segmented sum over each 8-elem (one byte of the row) segment.
        # Segment j's weights span 8 octaves (2^(31-8j)..2^(24-8j)), so
        # every partial sum is an integer multiple of 2^(24-8j) with
        # magnitude < 2^8 * 2^(24-8j): 8 significant bits => exact in bf16.
        nseg = F // 8
        e8 = e[:].rearrange("p (s x) -> p s x", x=8)
        s = spool.tile([P, nseg], dt.bfloat16, tag="s")
        if not cfg["tree"]:
            with nc.allow_low_precision(reason="segment sums exact in bf16"):
                eng(cfg["l1"]).tensor_reduce(
                    out=s[:], in_=e8, axis=mybir.AxisListType.X, op=Alu.add)
        else:
            t1 = pool.tile([P, nseg * 4], dt.bfloat16, tag="t1")
            t1v = t1[:].rearrange("p (s x) -> p s x", x=4)
            eng(cfg["l1"]).tensor_tensor(t1v, e8[:, :, 0:4], e8[:, :, 4:8],
                                         Alu.add)

            t2 = spool.tile([P, nseg * 2], dt.bfloat16, tag="t2")
            t2v = t2[:].rearrange("p (s x) -> p s x", x=2)
            eng(cfg["l2"]).tensor_tensor(t2v, t1v[:, :, 0:2], t1v[:, :, 2:4],
                                         Alu.add)

            sv = s[:].rearrange("p (s x) -> p s x", x=1)
            eng(cfg["l3"]).tensor_tensor(sv, t2v[:, :, 0:1], t2v[:, :, 1:2],
                                         Alu.add)

        # Combine the 4 per-row segment sums (already at their final
        # scales): V1 = s0 + s1 is exact in fp32 (terms span 2^16..2^31,
        # 16 significant bits), V0 = s2 + s3 <= 65535 exact, and
        # V = V1 + V0 is one rounded add of a multiple of 2^16 with a
        # < 2^16 term: sign exact, zero iff exact zero.
        s22 = s[:].rearrange("p (r u v) -> p r u v", u=2, v=2)
        vv = spool.tile([P, 2 * k], dt.float32, tag="vv")
        vv2 = vv[:].rearrange("p (r u) -> p r u", u=2)
        v = spool.tile([P, k], dt.float32, tag="v")
        e1 = eng(cfg["cmb"])
        e1.tensor_tensor(vv2, s22[:, :, :, 0:1], s22[:, :, :, 1:2], Alu.add)
        e1.tensor_tensor(v[:], vv2[:, :, 0:1], vv2[:, :, 1:2], Alu.add)

        # gt/eq into one interleaved [P, k, 2] tile -> single DMA store.
        odt = dt.bfloat16 if cfg["obf16"] else dt.float32
        ot = opool.tile([P, 2 * cfg["kmain"]], odt, tag="o")
        o = ot[:, :2 * k]
        o2 = o.rearrange("p (r c) -> p r c", c=2)
        e2 = eng(cfg["cmp"])
        e2.tensor_scalar(o2[:, :, 0:1], v[:], 0.0, None, Alu.is_gt)
        e2.tensor_scalar(o2[:, :, 1:2], v[:], 0.0, None, Alu.is_equal)

        eng(cfg["st"]).dma_start(
            out=o_dram[:].flatten()[2 * row0:2 * (row0 + rows)]
                .rearrange("(p r) -> p r", p=P),
            in_=o)
        row0 += rows
    assert row0 == ROWS_PER_CORE


def _legalize_waits(nc, mybir):
    """TRN2 ISA structs accept at most one sync wait per instruction (walrus
    codegen hard-errors otherwise). Tile's scheduler attaches one wait per
    dependency, so hoist all-but-one wait onto same-engine NoOps inserted
    immediately before; engines execute in order, so semantics are identical."""
    for fn in nc.m.functions:
        for blk in fn.blocks:
            new_insts = []
            for inst in blk.instructions:
                si = inst.sync_info
                waits = list(si.on_wait) if si is not None else []
                limit = 2 if isinstance(inst, mybir.InstEventSemaphore) else 1
                if len(waits) > limit:
                    for w in waits[:-limit]:
                        nop = mybir.InstNoOp(
                            name=nc.get_next_instruction_name(),
                            sync_info=mybir.SyncInfo(on_wait=[w], on_update=[]),
                            bass_nofuse=True,
                            engine=inst.engine,
                        )
                        nc.register_instruction(nop)
                        new_insts.append(nop)
                    si.on_wait = waits[-limit:]
                new_insts.append(inst)
            blk.instructions[:] = new_insts


DEFAULT_CFG = {"l1": "vector", "l2": "vector", "l3": "vector",
               "cmb": "vector", "cmp": "vector", "st": "scalar",
               "bufs": 3, "sbufs": 4, "obufs": 8, "abt": True,
               "subout": "inplace", "prescale": True, "tree": True,
               "obf16": False, "ramp": False, "kmain": K_MAIN}


def _build_program(repeat=1, dma_only=False, variant="b1", cfg=None):
    cfg = dict(DEFAULT_CFG, **(cfg or {}))
    key = ("nc", repeat, dma_only, variant, tuple(sorted(cfg.items())))
    if key in _CACHE:
        return _CACHE[key]

    from concourse.bass import Bass
    from concourse.tile import TileContext
    import concourse.mybir as mybir

    dt = mybir.dt

    nc = Bass(name="cmp32")
    if cfg["abt"] == "ilv":
        A = nc.dram_tensor("A", [ROWS_PER_CORE, 2 * BITS], dt.bfloat16,
                           kind="ExternalInput")
        B = A
    else:
        A = nc.dram_tensor("A", [ROWS_PER_CORE, BITS], dt.bfloat16,
                           kind="ExternalInput")
        B = nc.dram_tensor("B", [ROWS_PER_CORE, BITS], dt.bfloat16,
                           kind="ExternalInput")
    W = None if cfg["prescale"] else nc.dram_tensor(
        "W", [P, cfg["kmain"] * BITS], dt.bfloat16, kind="ExternalInput")
    O = nc.dram_tensor("O", [ROWS_PER_CORE, 2],
                       dt.bfloat16 if cfg["obf16"] else dt.float32,
                       kind="ExternalOutput")

    with TileContext(nc) as tc:
        with tc.tile_pool(name="wpool", bufs=1) as wpool, \
             tc.tile_pool(name="io", bufs=cfg["bufs"]) as pool, \
             tc.tile_pool(name="small", bufs=cfg["sbufs"]) as spool, \
             tc.tile_pool(name="opool", bufs=cfg["obufs"]) as opool:
            if cfg["prescale"]:
                wt = None
            else:
                wt = wpool.tile([P, cfg["kmain"] * BITS], dt.bfloat16)
                nc.sync.dma_start(out=wt[:], in_=W[:])

            for _rep in range(repeat):
                _emit_pass_b1(nc, pool, spool, opool, wt, A, B, O, mybir,
                              cfg, dma_only=dma_only)

    _legalize_waits(nc, mybir)
    _CACHE[key] = nc
    return nc


def _shard_inputs(A, B, cfg=None):
    """Split full inputs into 8 per-core maps (zero-pad only the last core).
    Values are {0,1}: exact in bfloat16, so inputs stream at half the bytes.
    With the interleaved layout, A and B rows are merged byte-wise into one
    [N, 64] array: [a0..7, b0..7, a8..15, b8..15, ...]."""
    import concourse.mybir as mybir
    cfg = dict(DEFAULT_CFG, **(cfg or {}))
    bf16 = mybir.dt.np(mybir.dt.bfloat16)
    w_tile = np.tile(_weight_row(), (P, cfg["kmain"])).astype(bf16)
    total = ROWS_PER_CORE * NCORES
    pad = total - N
    if cfg["prescale"]:
        w = _weight_row().astype(np.float32)
        A = (A * w).astype(bf16)
        B = (B * w).astype(bf16)
    else:
        A = A.astype(bf16)
        B = B.astype(bf16)
    ilv = cfg["abt"] == "ilv"
    if ilv:
        A = np.stack([A.reshape(N, 4, 8), B.reshape(N, 4, 8)],
                     axis=2).reshape(N, 64)
    width = A.shape[1]
    in_maps = []
    for c in range(NCORES):
        lo, hi = c * ROWS_PER_CORE, (c + 1) * ROWS_PER_CORE
        if hi <= N:
            a_sh, b_sh = A[lo:hi], B[lo:hi]
        else:
            z = np.zeros((pad, width), dtype=bf16)
            a_sh = np.concatenate([A[lo:N], z])
            if not ilv:
                b_sh = np.concatenate([B[lo:N], z])
        m = {"A": a_sh}
        if not cfg["prescale"]:
            m["W"] = w_tile
        if not ilv:
            m["B"] = b_sh
        in_maps.append(m)
    return in_maps


def kernel(A, B):
    from concourse.bass_utils import run_bass_kernel_spmd

    A = np.ascontiguousarray(A, dtype=np.float32)
    B = np.ascontiguousarray(B, dtype=np.float32)
    assert A.shape == (N, BITS) and B.shape == (N, BITS)

    nc = _build_program()
    in_maps = _shard_inputs(A, B)
    res = run_bass_kernel_spmd(nc, in_maps, core_ids=list(range(NCORES)))

    o = np.concatenate([r["O"] for r in res.results])[:N]
    og = np.ascontiguousarray(o[:, 0:1], dtype=np.float32)
    oe = np.ascontiguousarray(o[:, 1:2], dtype=np.float32)
    return og, oe


# revision 40
# speedup vs baseline: 1.0031x; 1.0031x over previous
"""Trainium2 Bass kernel: 32-bit soft-logic comparator (A > B, A == B).

Inputs A, B: [2_000_000, 32] float32 with values in {0.0, 1.0}, MSB first.
Outputs: (a_gt_b, a_eq_b), each [2_000_000, 1] float32 in {0.0, 1.0}.

Algorithm (exact, replaces the reference's prefix-product ladder):
  e_i = (a_i - b_i) * 2^(31-i)   in {0, +-2^(31-i)}
  V   = sum_i e_i  =  int32(A) - int32(B)
  a_gt_b = (V > 0), a_eq_b = (V == 0)
The sum is evaluated as a pairwise tree per 8-bit segment (every partial
sum inside segment j is an integer multiple of 2^(24-8j) with < 2^8
octave span => exactly representable in bf16), then the four segment
sums combine in fp32: V1 = s0+s1 (16 significant bits, exact),
V0 = s2+s3 (<= 65535, exact), V = V1+V0 (one rounded add of a multiple
of 2^16 plus a < 2^16 term: sign exact, zero iff exact zero).

Input staging: the values {0, 2^k} are exactly representable in
bfloat16, so the host encodes each input column in a fixed pre-scaled
bf16 fixed-point form (bit * 2^(31-i), a per-element lossless map).
The device then streams half the HBM bytes (memory-bound regime) and
the whole comparator pipeline (subtract, 3-level tree, combine,
compares) runs on-device on the DVE at its 2x bf16 rate. A plain
{0,1}-bf16 staging variant (device-side weight multiply) is kept behind
cfg["prescale"]=False.

Engine assignment: everything on DVE. Offloading tree levels to GpSimd
was measured and REGRESSES: concurrent GpSimd tensor ops slow DVE ops
sharing SBUF by 1.7x-9x (bank contention), and GpSimd small ops carry
~2us fixed overhead. gt/eq go into one interleaved [rows, 2] tile from
a dedicated deep pool (8 bufs) so store-queue backpressure never stalls
the compares; one DMA store per tile.

Sharding: data parallel along dim 0 across 8 cores, 250112 = 128*1954
rows/core; only the last core is zero-padded (896 rows), dropped on
gather.

Per-core layout: SBUF tiles [128, k*32] where partition p holds k
consecutive rows; 15 tiles of k=128 plus one tail tile of k=34.
Measured on trn2 (NTFF profile): ~105-126 us/core vs 91 us bf16 DMA
floor and 92 us DVE busy; the f32-input baseline was ~202-212 us.
"""

import numpy as np

N = 2_000_000
BITS = 32
NCORES = 8
P = 128
ROWS_PER_CORE = 250_112          # 128 * 1954 >= 2_000_000 / 8
GROUPS = ROWS_PER_CORE // P      # 1954 rows per partition per core
K_MAIN = 128
KS = [K_MAIN] * (GROUPS // K_MAIN) + ([GROUPS % K_MAIN] if GROUPS % K_MAIN else [])
SEG = 8                          # bits per exact-sum segment (b1 variant)

_CACHE = {}


def _weight_row():
    # w_i = 2^(31-i), MSB first; powers of two, exact in bf16.
    return 2.0 ** (31 - np.arange(BITS, dtype=np.float64))


def _ks(cfg):
    km = cfg["kmain"]
    ks = [km] * (GROUPS // km) + ([GROUPS % km] if GROUPS % km else [])
    if cfg["ramp"]:
        ks = [32, km - 32] + ks[1:]
    return ks


def _emit_pass_b1(nc, pool, spool, opool, wt, a_dram, b_dram, o_dram, mybir,
                  cfg, dma_only=False):
    """bf16 pipeline: sub (DVE 2x), 3-level pairwise-sum tree,
    combine, compares; one interleaved [rows, 2] store per tile."""
    dt = mybir.dt
    Alu = mybir.AluOpType

    def eng(name):
        return {"vector": nc.vector, "gpsimd": nc.gpsimd,
                "scalar": nc.scalar, "sync": nc.sync}[name]

    row0 = 0
    for k in _ks(cfg):
        rows = P * k
        F = k * BITS
        if cfg["abt"] == "rilv":
            # One [N, 64] host array with rows [a_row | b_row]: each tile
            # is a single contiguous HBM range -> one DMA with 16KB
            # descriptors per partition (best measured DMA efficiency).
            ab = pool.tile([P, 2 * F], dt.bfloat16, tag="ab")
            abv2 = ab[:].rearrange("p (r f) -> p r f", f=2 * BITS)
            a, b = abv2[:, :, 0:BITS], abv2[:, :, BITS:2 * BITS]
            iv = a_dram[:].flatten()[row0 * 2 * BITS:(row0 + rows) * 2 * BITS] \
                .rearrange("(p f) -> p f", p=P)
            nc.sync.dma_start(out=ab[:], in_=iv)
        elif cfg["abt"] == "ilv":
            # A and B arrive byte-interleaved from the host: row layout
            # [a0..7, b0..7, a8..15, b8..15, ...]. The subtract's operand
            # streams are then 16 B apart (the same access shape as the
            # fast tree adds), out is compact and in natural bit order.
            ab = pool.tile([P, 2 * F], dt.bfloat16, tag="ab")
            abv = ab[:].rearrange("p (s x) -> p s x", x=16)
            a, b = abv[:, :, 0:8], abv[:, :, 8:16]
            iv = a_dram[:].flatten()[row0 * 2 * BITS:(row0 + rows) * 2 * BITS] \
                .rearrange("(p f) -> p f", p=P)
            h2 = 2 * F // 2
            nc.sync.dma_start(out=ab[:, :h2], in_=iv[:, :h2])
            nc.scalar.dma_start(out=ab[:, h2:], in_=iv[:, h2:])
        else:
            av = a_dram[:].flatten()[row0 * BITS:(row0 + rows) * BITS] \
                .rearrange("(p f) -> p f", p=P)
            bv = b_dram[:].flatten()[row0 * BITS:(row0 + rows) * BITS] \
                .rearrange("(p f) -> p f", p=P)
            if cfg["abt"]:
                # a and b side by side in one tile: the subtract's two
                # operand streams stay address-adjacent in SBUF.
                ab = pool.tile([P, 2 * F], dt.bfloat16, tag="ab")
                a, b = ab[:, :F], ab[:, F:]
            else:
                at = pool.tile([P, F], dt.bfloat16, tag="a")
                bt = pool.tile([P, F], dt.bfloat16, tag="b")
                a, b = at[:], bt[:]
            nc.sync.dma_start(out=a, in_=av)
            nc.scalar.dma_start(out=b, in_=bv)
        if dma_only:
            row0 += rows
            continue

        # d = a - b in {-1,0,1}; e = d * w with w_i = 2^(31-i). Both bf16
        # exact (d*w is a signed power of two), DVE 2x rate. Writing d over
        # the a operand keeps the op at two SBUF streams (measured ~2.2us
        # vs ~3.6us for a third fresh stream).
        if cfg["prescale"]:
            # Host ships A*w and B*w (w_i = 2^(31-i), both exact bf16
            # encodings of the bits): the subtract directly yields the
            # weighted difference e.
            e = pool.tile([P, F], dt.bfloat16, tag="e")
            ev = e[:].rearrange("p (r f) -> p r f", f=BITS) \
                if cfg["abt"] == "rilv" else e[:]
            nc.vector.tensor_tensor(ev, a, b, Alu.subtract)
        else:
            if cfg["subout"] == "inplace" and cfg["abt"] is True:
                dd = a
            else:
                dt_ = pool.tile([P, F], dt.bfloat16, tag="d")
                dd = dt_[:]
            dv = dd.rearrange("p (s x) -> p s x", x=8) \
                if cfg["abt"] == "ilv" else dd
            nc.vector.tensor_tensor(dv, a, b, Alu.subtract)
            e = pool.tile([P, F], dt.bfloat16, tag="e")
            nc.vector.tensor_tensor(e[:], dd, wt[:, :F], Alu.mult)

        # Segmented sum over each 8-elem (one byte of the row) segment.
        # Segment j's weights span 8 octaves (2^(31-8j)..2^(24-8j)), so
        # every partial sum is an integer multiple of 2^(24-8j) with
        # magnitude < 2^8 * 2^(24-8j): 8 significant bits => exact in bf16.
        nseg = F // 8
        e8 = e[:].rearrange("p (s x) -> p s x", x=8)
        s = spool.tile([P, nseg], dt.bfloat16, tag="s")
        if not cfg["tree"]:
            with nc.allow_low_precision(reason="segment sums exact in bf16"):
                eng(cfg["l1"]).tensor_reduce(
                    out=s[:], in_=e8, axis=mybir.AxisListType.X, op=Alu.add)
        else:
            t1 = pool.tile([P, nseg * 4], dt.bfloat16, tag="t1")
            t1v = t1[:].rearrange("p (s x) -> p s x", x=4)
            eng(cfg["l1"]).tensor_tensor(t1v, e8[:, :, 0:4], e8[:, :, 4:8],
                                         Alu.add)

            t2 = spool.tile([P, nseg * 2], dt.bfloat16, tag="t2")
            t2v = t2[:].rearrange("p (s x) -> p s x", x=2)
            eng(cfg["l2"]).tensor_tensor(t2v, t1v[:, :, 0:2], t1v[:, :, 2:4],
                                         Alu.add)

            sv = s[:].rearrange("p (s x) -> p s x", x=1)
            eng(cfg["l3"]).tensor_tensor(sv, t2v[:, :, 0:1], t2v[:, :, 1:2],
                                         Alu.add)

        # Combine the 4 per-row segment sums (already at their final
        # scales): V1 = s0 + s1 is exact in fp32 (terms span 2^16..2^31,
        # 16 significant bits), V0 = s2 + s3 <= 65535 exact, and
        # V = V1 + V0 is one rounded add of a multiple of 2^16 with a
        # < 2^16 term: sign exact, zero iff exact zero.
        v = spool.tile([P, k], dt.float32, tag="v")
        if cfg["cmbred"]:
            # Single segmented reduce of the 4 per-row sums: every partial
            # order of (s0+s1+s2)+s3 stays exact except one final sign-safe
            # rounded add (multiples of 2^8/2^16 plus sub-2^8/2^16 terms).
            s4r = s[:].rearrange("p (r j) -> p r j", j=4)
            nc.vector.tensor_reduce(out=v[:], in_=s4r,
                                    axis=mybir.AxisListType.X, op=Alu.add)
        else:
            s22 = s[:].rearrange("p (r u v) -> p r u v", u=2, v=2)
            vv = spool.tile([P, 2 * k], dt.float32, tag="vv")
            vv2 = vv[:].rearrange("p (r u) -> p r u", u=2)
            e1 = eng(cfg["cmb"])
            e1.tensor_tensor(vv2, s22[:, :, :, 0:1], s22[:, :, :, 1:2],
                             Alu.add)
            e1.tensor_tensor(v[:], vv2[:, :, 0:1], vv2[:, :, 1:2], Alu.add)

        # gt/eq into one interleaved [P, k, 2] tile -> single DMA store.
        odt = dt.bfloat16 if cfg["obf16"] else dt.float32
        ot = opool.tile([P, 2 * cfg["kmain"]], odt, tag="o")
        o = ot[:, :2 * k]
        o2 = o.rearrange("p (r c) -> p r c", c=2)
        e2 = eng(cfg["cmp"])
        e2.tensor_scalar(o2[:, :, 0:1], v[:], 0.0, None, Alu.is_gt)
        e2.tensor_scalar(o2[:, :, 1:2], v[:], 0.0, None, Alu.is_equal)

        eng(cfg["st"]).dma_start(
            out=o_dram[:].flatten()[2 * row0:2 * (row0 + rows)]
                .rearrange("(p r) -> p r", p=P),
            in_=o)
        row0 += rows
    assert row0 == ROWS_PER_CORE


def _legalize_waits(nc, mybir):
    """TRN2 ISA structs accept at most one sync wait per instruction (walrus
    codegen hard-errors otherwise). Tile's scheduler attaches one wait per
    dependency, so hoist all-but-one wait onto same-engine NoOps inserted
    immediately before; engines execute in order, so semantics are identical."""
    for fn in nc.m.functions:
        for blk in fn.blocks:
            new_insts = []
            for inst in blk.instructions:
                si = inst.sync_info
                waits = list(si.on_wait) if si is not None else []
                limit = 2 if isinstance(inst, mybir.InstEventSemaphore) else 1
                if len(waits) > limit:
                    for w in waits[:-limit]:
                        nop = mybir.InstNoOp(
                            name=nc.get_next_instruction_name(),
                            sync_info=mybir.SyncInfo(on_wait=[w], on_update=[]),
                            bass_nofuse=True,
                            engine=inst.engine,
                        )
                        nc.register_instruction(nop)
                        new_insts.append(nop)
                    si.on_wait = waits[-limit:]
                new_insts.append(inst)
            blk.instructions[:] = new_insts


DEFAULT_CFG = {"l1": "vector", "l2": "vector", "l3": "vector",
               "cmb": "vector", "cmp": "vector", "st": "scalar",
               "bufs": 3, "sbufs": 4, "obufs": 8, "abt": True,
               "subout": "inplace", "prescale": True, "tree": True,
               "obf16": False, "ramp": False, "kmain": K_MAIN, "cmbred": False}


def _build_program(repeat=1, dma_only=False, variant="b1", cfg=None):
    cfg = dict(DEFAULT_CFG, **(cfg or {}))
    key = ("nc", repeat, dma_only, variant, tuple(sorted(cfg.items())))
    if key in _CACHE:
        return _CACHE[key]

    from concourse.bass import Bass
    from concourse.tile import TileContext
    import concourse.mybir as mybir

    dt = mybir.dt

    nc = Bass(name="cmp32")
    if cfg["abt"] in ("ilv", "rilv"):
        A = nc.dram_tensor("A", [ROWS_PER_CORE, 2 * BITS], dt.bfloat16,
                           kind="ExternalInput")
        B = A
    else:
        A = nc.dram_tensor("A", [ROWS_PER_CORE, BITS], dt.bfloat16,
                           kind="ExternalInput")
        B = nc.dram_tensor("B", [ROWS_PER_CORE, BITS], dt.bfloat16,
                           kind="ExternalInput")
    W = None if cfg["prescale"] else nc.dram_tensor(
        "W", [P, cfg["kmain"] * BITS], dt.bfloat16, kind="ExternalInput")
    O = nc.dram_tensor("O", [ROWS_PER_CORE, 2],
                       dt.bfloat16 if cfg["obf16"] else dt.float32,
                       kind="ExternalOutput")

    with TileContext(nc) as tc:
        with tc.tile_pool(name="wpool", bufs=1) as wpool, \
             tc.tile_pool(name="io", bufs=cfg["bufs"]) as pool, \
             tc.tile_pool(name="small", bufs=cfg["sbufs"]) as spool, \
             tc.tile_pool(name="opool", bufs=cfg["obufs"]) as opool:
            if cfg["prescale"]:
                wt = None
            else:
                wt = wpool.tile([P, cfg["kmain"] * BITS], dt.bfloat16)
                nc.sync.dma_start(out=wt[:], in_=W[:])

            for _rep in range(repeat):
                _emit_pass_b1(nc, pool, spool, opool, wt, A, B, O, mybir,
                              cfg, dma_only=dma_only)

    _legalize_waits(nc, mybir)
    _CACHE[key] = nc
    return nc


def _shard_inputs(A, B, cfg=None):
    """Split full inputs into 8 per-core maps (zero-pad only the last core).
    Values are {0,1}: exact in bfloat16, so inputs stream at half the bytes.
    With the interleaved layout, A and B rows are merged byte-wise into one
    [N, 64] array: [a0..7, b0..7, a8..15, b8..15, ...]."""
    import concourse.mybir as mybir
    cfg = dict(DEFAULT_CFG, **(cfg or {}))
    bf16 = mybir.dt.np(mybir.dt.bfloat16)
    w_tile = np.tile(_weight_row(), (P, cfg["kmain"])).astype(bf16)
    total = ROWS_PER_CORE * NCORES
    pad = total - N
    if cfg["prescale"]:
        w = _weight_row().astype(np.float32)
        A = (A * w).astype(bf16)
        B = (B * w).astype(bf16)
    else:
        A = A.astype(bf16)
        B = B.astype(bf16)
    ilv = cfg["abt"] in ("ilv", "rilv")
    if cfg["abt"] == "ilv":
        A = np.stack([A.reshape(N, 4, 8), B.reshape(N, 4, 8)],
                     axis=2).reshape(N, 64)
    elif cfg["abt"] == "rilv":
        A = np.concatenate([A.reshape(N, 1, BITS), B.reshape(N, 1, BITS)],
                           axis=1).reshape(N, 2 * BITS)
    width = A.shape[1]
    in_maps = []
    for c in range(NCORES):
        lo, hi = c * ROWS_PER_CORE, (c + 1) * ROWS_PER_CORE
        if hi <= N:
            a_sh, b_sh = A[lo:hi], B[lo:hi]
        else:
            z = np.zeros((pad, width), dtype=bf16)
            a_sh = np.concatenate([A[lo:N], z])
            if not ilv:
                b_sh = np.concatenate([B[lo:N], z])
        m = {"A": a_sh}
        if not cfg["prescale"]:
            m["W"] = w_tile
        if not ilv:
            m["B"] = b_sh
        in_maps.append(m)
    return in_maps


def kernel(A, B):
    from concourse.bass_utils import run_bass_kernel_spmd

    A = np.ascontiguousarray(A, dtype=np.float32)
    B = np.ascontiguousarray(B, dtype=np.float32)
    assert A.shape == (N, BITS) and B.shape == (N, BITS)

    nc = _build_program()
    in_maps = _shard_inputs(A, B)
    res = run_bass_kernel_spmd(nc, in_maps, core_ids=list(range(NCORES)))

    o = np.concatenate([r["O"] for r in res.results])[:N]
    og = np.ascontiguousarray(o[:, 0:1], dtype=np.float32)
    oe = np.ascontiguousarray(o[:, 1:2], dtype=np.float32)
    return og, oe
